# revision 34
# baseline (speedup 1.0000x reference)
"""Trainium2 Bass kernel for tf-idf embedding pooling + MLP (v7).

Math identity: pooled[b] = sum_v c_{b,v}^2 * idf_v * emb_v where c = per-row
token counts, idf is folded into the embedding table on host
(emb*idf), and v = hi*128 + lo.

Sharding: by VOCAB, not batch. The host routes each token's one-hot pair
(lo in [0,128), hi_local in [0,49)) to the core owning its hi-range
(8 shards x 49 hi bins). Each core builds count matrices for ALL 64 batch
rows over ITS 6272-row vocab shard directly in matmul-ready layout
(H[lo, hl, b], via one-hot matmuls onehot_lo^T @ onehot_hi with two batch
rows sharing each matmul's 98 rhs columns), squares them (counts are exact
small ints), and contracts its idf-prescaled bf16 emb shard into partial
pooled[64, 256]. A single tiny AllToAll (32KB) redistributes the partials
so each core tree-adds its own 8 rows and runs the MLP + softmax.

This eliminates the 800KB counts AllToAll of the batch-sharded design
(measured 27-35us per rep under DMA load - the collective SDMA shares DMA
engines with regular traffic). One-hots are precomputed on host in fp8:
on-device DVE is_equal generation measured 10-30x slower than nominal.
The rep loop is software-pipelined 2 deep so rep k's partial-a2a and MLP
tail hide under rep k+1's histogram.
"""

import sys

import numpy as np

sys.path.insert(0, "/opt/trn_rl_repo")

import concourse.bass as bass  # noqa: E402,F401
import concourse.mybir as mybir  # noqa: E402
import concourse.tile as tile  # noqa: E402
from concourse import bacc  # noqa: E402
from concourse.masks import make_identity  # noqa: E402

P = 128
S = 2048
B = 64
D = 256
V = 50000
NCORES = 8
RPC = B // NCORES  # 8 output rows per core
NLO = 128
NHI = 392  # 391 real hi values + 1 pad
NHL = NHI // NCORES  # 49 hi bins per vocab shard
VSH = NHL * NLO  # 6272 vocab rows per shard
VPAD = NHI * NLO  # 50176
NPAIR = B // 2  # 32 row-pairs, two rows share each matmul's rhs columns
PW = 2 * NHL  # 98 rhs columns per matmul (2 rows x 49 hi bins)
PT = 5  # tiles (of 128 token slots) per (core, row-pair); max seen 573
PBLK = 4  # row-pairs per PSUM bank tile (4*98 = 392 fp32 cols)
NBLK = NPAIR // PBLK  # 8 PSUM tiles per rep


F32 = mybir.dt.float32
BF16 = mybir.dt.bfloat16
F8 = mybir.dt.float8e4

_CACHE = {}


def _mlp_tail(nc, tc, cpool, ps_mlp, pooled_sb, identity,
              w1t_sb, b1_sb, w2t_sb, b2a_sb, b2b_sb, w3a_sb, w3b_sb, b3_sb, out):
    """pooled_sb [RPC, 256] f32 -> softmax out DMA.

    All PSUM intermediates are disjoint column slices of one bank tile
    (PSUM pool allocation is bank-granular)."""
    mlp_ps = ps_mlp.tile([P, 64], F32, tag="mlp")
    pooledT = cpool.tile([P, 2, RPC], F32, tag="pooledT", bufs=2)
    for kc in range(2):
        ptp = mlp_ps[:, kc * RPC : (kc + 1) * RPC]
        nc.tensor.transpose(
            ptp, pooled_sb[:, kc * P : (kc + 1) * P], identity[:RPC, :RPC]
        )
        nc.vector.tensor_copy(pooledT[:, kc, :], ptp)

    h1_ps = mlp_ps[:100, 16:24]
    for kc in range(2):
        nc.tensor.matmul(
            h1_ps, lhsT=w1t_sb[:, kc, :], rhs=pooledT[:, kc, :],
            start=(kc == 0), stop=(kc == 1),
        )
    h1_sb = cpool.tile([100, RPC], F32, tag="h1_sb", bufs=2)
    nc.scalar.activation(
        h1_sb[:], h1_ps, mybir.ActivationFunctionType.Relu,
        bias=b1_sb[:, 0:1], scale=1.0,
    )

    h2a_ps = mlp_ps[:, 24:32]
    nc.tensor.matmul(h2a_ps, lhsT=w2t_sb[:, 0:128], rhs=h1_sb[:, :],
                     start=True, stop=True)
    h2b_ps = mlp_ps[:22, 32:40]
    nc.tensor.matmul(h2b_ps, lhsT=w2t_sb[:, 128:150], rhs=h1_sb[:, :],
                     start=True, stop=True)
    h2a_sb = cpool.tile([P, RPC], F32, tag="h2a_sb", bufs=2)
    h2b_sb = cpool.tile([22, RPC], F32, tag="h2b_sb", bufs=2)
    nc.scalar.activation(h2a_sb[:], h2a_ps,
                         mybir.ActivationFunctionType.Relu,
                         bias=b2a_sb[:, 0:1], scale=1.0)
    nc.scalar.activation(h2b_sb[:], h2b_ps,
                         mybir.ActivationFunctionType.Relu,
                         bias=b2b_sb[:, 0:1], scale=1.0)

    lg_ps = mlp_ps[:2, 40:48]
    nc.tensor.matmul(lg_ps, lhsT=w3a_sb[:, :], rhs=h2a_sb[:, :],
                     start=True, stop=False)
    nc.tensor.matmul(lg_ps, lhsT=w3b_sb[:, :], rhs=h2b_sb[:, :],
                     start=False, stop=True)
    lg_sb = cpool.tile([2, RPC], F32, tag="lg_sb", bufs=2)
    nc.scalar.add(lg_sb[:], lg_ps, b3_sb[:, 0:1])

    lt_ps = mlp_ps[:RPC, 48:50]
    nc.tensor.transpose(lt_ps, lg_sb[:, :], identity[:2, :2])
    e_sb = cpool.tile([RPC, 2], F32, tag="e_sb", bufs=2)
    nc.scalar.activation(e_sb[:], lt_ps[:, :], mybir.ActivationFunctionType.Exp)
    ssum = cpool.tile([RPC, 1], F32, tag="ssum", bufs=2)
    nc.vector.tensor_reduce(ssum[:], e_sb[:], axis=mybir.AxisListType.X,
                            op=mybir.AluOpType.add)
    rinv = cpool.tile([RPC, 1], F32, tag="rinv", bufs=2)
    nc.vector.reciprocal(rinv[:], ssum[:])
    res_sb = cpool.tile([RPC, 2], F32, tag="res_sb", bufs=2)
    nc.vector.tensor_scalar(out=res_sb[:], in0=e_sb[:], scalar1=rinv[:, 0:1],
                            scalar2=None, op0=mybir.AluOpType.mult)
    nc.scalar.dma_start(out[:, :], res_sb[:])


def _build_nc(reps=1):
    nc = bacc.Bacc(None, target_bir_lowering=False, debug=False)

    ohlo = nc.dram_tensor("ohlo", [P, NPAIR, PT, NLO], F8, kind="ExternalInput")
    ohhi = nc.dram_tensor("ohhi", [P, NPAIR, PT, PW], F8, kind="ExternalInput")
    # pre-packed on host as [p, hl, d] so each partition's 49x256 block is one
    # contiguous 25KB DMA run
    embs = nc.dram_tensor("embs", [P, NHL, D], BF16, kind="ExternalInput")
    w1t = nc.dram_tensor("w1t", [256, 100], F32, kind="ExternalInput")
    b1 = nc.dram_tensor("b1", [100], F32, kind="ExternalInput")
    w2t = nc.dram_tensor("w2t", [100, 150], F32, kind="ExternalInput")
    b2 = nc.dram_tensor("b2", [150], F32, kind="ExternalInput")
    w3t = nc.dram_tensor("w3t", [150, 2], F32, kind="ExternalInput")
    b3 = nc.dram_tensor("b3", [2], F32, kind="ExternalInput")
    out = nc.dram_tensor("out", [RPC, 2], F32, kind="ExternalOutput")

    with tile.TileContext(nc) as tc:
        with (
            tc.tile_pool(name="const", bufs=1) as cpool,
            tc.tile_pool(name="oh", bufs=3) as lopool,
            tc.tile_pool(name="emb", bufs=2) as epool,
            tc.tile_pool(name="work", bufs=2) as wpool,
            tc.tile_pool(name="dram", bufs=2, space="DRAM") as dpool,
            tc.tile_pool(name="ps_ht", bufs=2, space="PSUM") as ps_ht,
            tc.tile_pool(name="ps_acc", bufs=2, space="PSUM") as ps_acc,
            tc.tile_pool(name="ps_mlp", bufs=1, space="PSUM") as ps_mlp,
        ):
            # ---------- constants (amortized across reps) ----------
            identity = cpool.tile([P, P], F32)
            make_identity(nc, identity[:])

            w1t_sb = cpool.tile([P, 2, 100], F32)
            nc.sync.dma_start(w1t_sb[:, :, :],
                              w1t[:, :].rearrange("(c p) m -> p c m", p=P))
            b1_sb = cpool.tile([100, 1], F32)
            nc.sync.dma_start(b1_sb[:, :], b1[:, None])
            w2t_sb = cpool.tile([100, 150], F32)
            nc.sync.dma_start(w2t_sb[:, :], w2t[:, :])
            b2a_sb = cpool.tile([128, 1], F32)
            b2b_sb = cpool.tile([22, 1], F32)
            nc.sync.dma_start(b2a_sb[:, :], b2[:128, None])
            nc.sync.dma_start(b2b_sb[:, :], b2[128:150, None])
            w3a_sb = cpool.tile([128, 2], F32)
            w3b_sb = cpool.tile([22, 2], F32)
            nc.sync.dma_start(w3a_sb[:, :], w3t[0:128, :])
            nc.sync.dma_start(w3b_sb[:, :], w3t[128:150, :])
            b3_sb = cpool.tile([2, 1], F32)
            nc.sync.dma_start(b3_sb[:, :], b3[:, None])

            def emit_front_dma(rep):
                """Bulk one-hot + emb DMAs, issued one pipeline stage ahead
                of the compute so the DMA queues never drain dry at rep
                boundaries."""
                lo_sb = lopool.tile([P, NPAIR, PT, NLO], F8, tag="lo")
                hi_sb = lopool.tile([P, NPAIR, PT, PW], F8, tag="hi")
                nc.sync.dma_start(lo_sb[:, :, :, :], ohlo[:, :, :, :])
                nc.sync.dma_start(hi_sb[:, :, :, :], ohhi[:, :, :, :])

                emb_sb = epool.tile([P, NHL, D], BF16, tag="emb_sb")
                nc.sync.dma_start(emb_sb[:, :, :], embs[:, :, :])
                return {"lo": lo_sb, "hi": hi_sb, "emb": emb_sb}

            def emit_front_compute(dm):
                """Histogram + square + pooled matmul + tiny a2a carrying
                this rep's pooled partials."""
                lo_sb, hi_sb, emb_sb = dm["lo"], dm["hi"], dm["emb"]
                # count matrices for all 64 rows, matmul-ready [p, hl, b]
                a_mm = wpool.tile([P, NHL, B], BF16, tag="a_mm")
                for blk in range(NBLK):  # 4 row-pairs (8 rows) per PSUM bank
                    ht_ps = ps_ht.tile([P, PBLK * PW], F32, tag="ht")
                    for j in range(PBLK):
                        pair = blk * PBLK + j
                        for f in range(PT):
                            nc.tensor.matmul(
                                ht_ps[:, j * PW : (j + 1) * PW],
                                lhsT=lo_sb[:, pair, f, :],
                                rhs=hi_sb[:, pair, f, :],
                                start=(f == 0), stop=(f == PT - 1))
                    # square the counts fused with the [p, hl, b] reshuffle
                    # (ACT only: DVE can't read both operands from PSUM)
                    sq_in = ht_ps[:, :].rearrange(
                        "p (j rp hl) -> p hl (j rp)", j=PBLK, rp=2)
                    dst = a_mm[:, :, blk * 8 : (blk + 1) * 8]
                    nc.scalar.activation(
                        dst, sq_in,
                        mybir.ActivationFunctionType.Square, scale=1.0)

                pooled_ps = ps_acc.tile([B, D], F32, tag="pooled")
                for c in range(NHL):  # 49 chunks of 128 vocab rows
                    nc.tensor.matmul(
                        pooled_ps[:, :],
                        lhsT=a_mm[:, c, :],
                        rhs=emb_sb[:, c, :],
                        start=(c == 0), stop=(c == NHL - 1))
                pooled_bf = wpool.tile([B, D], BF16, tag="pooled_bf")
                nc.vector.tensor_copy(pooled_bf[:], pooled_ps[:, :])

                pa_in = dpool.tile([NCORES, RPC * D], BF16, tag="pa_in")
                pa_out = dpool.tile([NCORES, RPC * D], BF16, tag="pa_out")
                pin3 = pa_in[:, :].rearrange("dst (r d) -> dst r d", r=RPC)
                for dst in range(NCORES):
                    nc.scalar.dma_start(
                        pin3[dst], pooled_bf[dst * RPC : (dst + 1) * RPC, :])
                nc.gpsimd.collective_compute(
                    "AllToAll", mybir.AluOpType.bypass,
                    replica_groups=[list(range(NCORES))],
                    ins=[pa_in[:, :]],
                    outs=[pa_out[:, :]],
                )
                return {"pa_out": pa_out}

            def emit_tail(st):
                """Sum the 8 pooled partial blocks, then MLP + softmax."""
                pt = wpool.tile([RPC, NCORES, D], BF16, tag="pt")
                nc.scalar.dma_start(
                    pt[:, :, :],
                    st["pa_out"][:, :].rearrange("src (r d) -> r src d", r=RPC),
                )
                s1 = wpool.tile([RPC, 4, D], F32, tag="s1")
                nc.vector.tensor_tensor(out=s1[:, :, :], in0=pt[:, 0:4, :],
                                        in1=pt[:, 4:8, :],
                                        op=mybir.AluOpType.add)
                s2 = wpool.tile([RPC, 2, D], F32, tag="s2")
                nc.vector.tensor_tensor(out=s2[:, :, :], in0=s1[:, 0:2, :],
                                        in1=s1[:, 2:4, :],
                                        op=mybir.AluOpType.add)
                pooled_sb = wpool.tile([RPC, D], F32, tag="pooled_sb")
                nc.vector.tensor_tensor(out=pooled_sb[:, :], in0=s2[:, 0, :],
                                        in1=s2[:, 1, :],
                                        op=mybir.AluOpType.add)
                _mlp_tail(nc, tc, cpool, ps_mlp, pooled_sb, identity,
                          w1t_sb, b1_sb, w2t_sb, b2a_sb, b2b_sb,
                          w3a_sb, w3b_sb, b3_sb, out)

            dmas = []
            sts = []
            for rep in range(reps):
                dmas.append(emit_front_dma(rep))
                if rep >= 1:
                    sts.append(emit_front_compute(dmas[rep - 1]))
                if rep >= 2:
                    emit_tail(sts[rep - 2])
            sts.append(emit_front_compute(dmas[-1]))
            if reps >= 2:
                emit_tail(sts[-2])
            emit_tail(sts[-1])

    nc.compile()
    return nc


def make_in_maps(x, emb, idf, W1, b1, W2, b2, W3, b3):
    bf16 = mybir.dt.np(BF16)
    f8 = mybir.dt.np(F8)

    xt = np.asarray(x, dtype=np.int64).T  # [B, S]

    idf_pad = np.zeros(VPAD, dtype=np.float32)
    idf_pad[:V] = np.asarray(idf, dtype=np.float32)
    idf_pad[0] = 0.0  # pad token contributes nothing

    # fold idf into the embedding table: pooled = sum_v H_v^2 (idf_v emb_v)
    emb_pad = np.zeros((VPAD, D), dtype=np.float32)
    emb_pad[:V] = np.asarray(emb, dtype=np.float32)
    emb_pad *= idf_pad[:, None]
    emb_bf16 = emb_pad.astype(bf16)

    w1t = np.ascontiguousarray(np.asarray(W1, dtype=np.float32).T)
    w2t = np.ascontiguousarray(np.asarray(W2, dtype=np.float32).T)
    w3t = np.ascontiguousarray(np.asarray(W3, dtype=np.float32).T)
    b1 = np.ascontiguousarray(np.asarray(b1, dtype=np.float32))
    b2 = np.ascontiguousarray(np.asarray(b2, dtype=np.float32))
    b3 = np.ascontiguousarray(np.asarray(b3, dtype=np.float32))

    lo_all = (xt & (NLO - 1)).astype(np.int64)  # [B, S]
    hi_all = (xt >> 7).astype(np.int64)
    shard_all = hi_all // NHL

    in_maps = []
    one = f8(1.0)
    for c in range(NCORES):
        ohlo = np.zeros((P, NPAIR, PT, NLO), dtype=f8)
        ohhi = np.zeros((P, NPAIR, PT, PW), dtype=f8)
        bb, ss = np.nonzero(shard_all == c)  # sorted by (b, s)
        pair = bb // 2
        rho = bb & 1
        starts = np.searchsorted(pair, np.arange(NPAIR))
        rank = np.arange(len(bb)) - starts[pair]
        assert rank.max() < PT * P, "row-pair shard overflow: raise PT"
        p = rank % P
        f = rank // P
        ohlo[p, pair, f, lo_all[bb, ss]] = one
        ohhi[p, pair, f, rho * NHL + (hi_all[bb, ss] - c * NHL)] = one

        eshard = emb_bf16[c * VSH : (c + 1) * VSH]  # [6272, 256]
        epacked = np.ascontiguousarray(
            eshard.reshape(NHL, P, D).transpose(1, 0, 2))  # [128, 49, 256]
        m = {
            "ohlo": ohlo,
            "ohhi": ohhi,
            "embs": epacked,
            "w1t": w1t, "b1": b1, "w2t": w2t, "b2": b2,
            "w3t": w3t, "b3": b3,
        }
        in_maps.append(m)
    return in_maps


def _get_nc(reps=1):
    key = f"nc9_r{reps}"
    if key not in _CACHE:
        _CACHE[key] = _build_nc(reps)
    return _CACHE[key]


class _Runner:
    """Cached jitted shard_map over the NEFF custom call (mirrors
    bass2jax.run_bass_via_pjrt, but reusable with device-resident inputs)."""

    def __init__(self, nc):
        import jax
        from jax.experimental.shard_map import shard_map
        from jax.sharding import Mesh, NamedSharding, PartitionSpec

        from concourse import bass2jax

        bass2jax.install_neuronx_cc_hook()
        assert nc.dbg_addr is None
        partition_name = (
            nc.partition_id_tensor.name if nc.partition_id_tensor else None
        )
        self._nc = nc
        self._partition_name = partition_name

        self.jax = jax
        in_names, out_names, out_avals, zero_outs = [], [], [], []
        for alloc in nc.m.functions[0].allocations:
            if not isinstance(alloc, mybir.MemoryLocationSet):
                continue
            name = alloc.memorylocations[0].name
            if alloc.kind == "ExternalInput":
                if name == partition_name:
                    continue
                in_names.append(name)
            elif alloc.kind == "ExternalOutput":
                out_names.append(name)
                shape = tuple(alloc.tensor_shape)
                dtype = mybir.dt.np(alloc.dtype)
                out_avals.append(jax.core.ShapedArray(shape, dtype))
                zero_outs.append(np.zeros((NCORES * shape[0], *shape[1:]), dtype))
        self.in_names = list(in_names)
        self.out_names = out_names
        self.out_avals = out_avals
        self.zero_outs = zero_outs
        n_params = len(in_names)
        n_outs = len(out_names)
        bind_names = tuple(
            in_names + out_names + ([partition_name] if partition_name else [])
        )
        donate = tuple(range(n_params, n_params + n_outs))

        def _body(*args):
            operands = list(args)
            if partition_name is not None:
                operands.append(bass2jax.partition_id_tensor())
            outs = bass2jax._bass_exec_p.bind(
                *operands,
                out_avals=tuple(out_avals),
                in_names=bind_names,
                out_names=tuple(out_names),
                lowering_input_output_aliases=(),
                sim_require_finite=True,
                sim_require_nnan=True,
                nc=nc,
            )
            return tuple(outs)

        devices = jax.devices()[:NCORES]
        self.mesh = Mesh(np.asarray(devices), ("core",))
        self.sharding = NamedSharding(self.mesh, PartitionSpec("core"))
        in_specs = (PartitionSpec("core"),) * (n_params + n_outs)
        out_specs = (PartitionSpec("core"),) * n_outs
        self.fn = jax.jit(
            shard_map(
                _body,
                mesh=self.mesh,
                in_specs=in_specs,
                out_specs=out_specs,
                check_rep=False,
            ),
            donate_argnums=donate,
            keep_unused=True,
        )

    def put_inputs(self, in_maps):
        concat = [
            np.concatenate([np.asarray(m[name]) for m in in_maps], axis=0)
            for name in self.in_names
        ]
        return [self.jax.device_put(a, self.sharding) for a in concat]

    def run(self, dev_in):
        zo = [self.jax.device_put(z, self.sharding) for z in self.zero_outs]
        outs = self.fn(*dev_in, *zo)
        self.jax.block_until_ready(outs)
        return outs

    def run_np(self, dev_in):
        outs = self.run(dev_in)
        return {
            name: np.asarray(outs[i]).reshape(NCORES, *self.out_avals[i].shape)
            for i, name in enumerate(self.out_names)
        }


def _get_runner(reps=1):
    key = f"runner9_r{reps}"
    if key not in _CACHE:
        _CACHE[key] = _Runner(_get_nc(reps))
    return _CACHE[key]


def kernel(x, emb, idf, W1, b1, W2, b2, W3, b3):
    in_maps = make_in_maps(x, emb, idf, W1, b1, W2, b2, W3, b3)
    runner = _get_runner(1)
    dev_in = runner.put_inputs(in_maps)
    outs = runner.run_np(dev_in)
    outp = np.concatenate([outs["out"][c] for c in range(NCORES)], axis=0)
    return outp.astype(np.float32)
